# revision 49
# baseline (speedup 1.0000x reference)
"""GatedGraphNN Trainium2 kernel: 8-core SPMD, node-sharded, overlapped AllGather.

Algorithm notes:
  - messages = h[col] @ W_msg.T + b_msg ; agg = segsum(messages, row). Linearity:
    agg @ W_ih.T = raw @ (W_ih W_msg).T + outer(deg, W_ih b_msg), raw = segsum(h[col]).
    So no per-edge matmul: gather h[col] (bf16), segment-sum via one-hot matmuls on PE,
    then fused dense GRU with W_c = W_ih @ W_msg.
  - Each core owns 12800 dest nodes (N padded 100000->102400). Edges bucketed by
    (dest-tile, src-residue, dest-quarter) cells padded to 32-slot granularity (size
    shared across cores: SPMD-uniform program); segsum matmuls partition-slice cells
    that cross 128-slot g columns (PE base partition must be 0/32/64, so cell starts
    avoid 96 mod 128 via spacers). ~21% slot padding vs 36% at 128 granularity.
  - Gathers use 4 SWDGE queues (queue_num = source residue r): descriptor generation
    runs on 4 Q7 core-pairs concurrently, ~3x the single-queue rate.
  - h kept bf16: node-major replica h_full [102400,128] in DRAM for gathers (step 0
    reads the host-provided full x, so there is no initial exchange); feature-major
    shard resident in SBUF for the W_hh matmul.
  - The per-step AllGather is split into geometric chunks (CHB) fired as their dest
    tiles finish, overlapping the exchange with compute; h_full and bounce are
    double-buffered across steps to avoid gather-vs-exchange races. Node rows are
    permuted chunk-major (see _perm) so each chunk's output is contiguous.
  - Epilogue (gates, blend) computed with fp32 intermediates; h state rounds to bf16
    once per step. Final step emits fp32.
"""

import numpy as np
import ml_dtypes

BF16 = ml_dtypes.bfloat16
N, H, STEPS, NCORES = 100000, 128, 5, 8
NPAD = 102400
SHARD = NPAD // NCORES          # 12800
NT = SHARD // 512               # 25 dense tiles of 512 dests
NQ = SHARD // 128               # 100 quarters of 128 dests
CHUNK = 128
# AllGather chunk boundaries (in dest tiles): decreasing sizes so late chunks
# still hide under remaining compute; h_full rows are chunk-major then core.
# Each chunk's AllGather is issued 2 tiles after the chunk completes so the
# issue's sem wait (on bounce writes) doesn't stall the gather stream.
CHB = [0, 2, 6, 11, 17, 25]
NCH = len(CHB) - 1
AG_DELAY = 2


def _perm():
    """P[n] = physical row of logical node n in h_full (chunk-major layout)."""
    n = np.arange(NPAD, dtype=np.int64)
    c, l = n // SHARD, n % SHARD
    t, o = l // 512, l % 512
    chb = np.asarray(CHB)
    ch = np.searchsorted(chb, t, side="right") - 1
    base = 8 * 512 * chb[ch]
    size = 512 * (chb[ch + 1] - chb[ch])
    return base + c * size + (t - chb[ch]) * 512 + o


def _preprocess(edge_index):
    """Host-side tables, 32-granular cells. Slots grouped per gather call (T, r)
    with the 4 dest-qtr cells at 32-aligned offsets inside the call; no 128
    alignment, so segsum matmuls partition-slice cells that cross g columns.
    idx value = pcol//4 (stride-4 table view), pcol = permuted source row."""
    row = np.asarray(edge_index[0]).astype(np.int64)
    col = np.asarray(edge_index[1]).astype(np.int64)
    col = _perm()[col]
    core = row // SHARD
    rloc = row - core * SHARD
    T = rloc // 512
    dq = (rloc % 512) // 128
    off = rloc % 128
    res = col % 4
    cell = (T * 4 + res) * 4 + dq          # (T, res, dq) -> 0..399
    NCELL = NT * 16
    cnt = np.zeros((NCORES, NCELL), np.int64)
    np.add.at(cnt, (core, cell), 1)
    cs = np.maximum(32, ((cnt.max(axis=0) + 31) // 32) * 32)   # slots per cell
    # cell start offsets within each call; PE base-partition must be 0/32/64,
    # so a cell may not start at 96 mod 128 -> insert a 32-slot spacer there
    cell_a = np.zeros(NCELL, np.int64)     # call-local start of each cell
    call_s = np.zeros((NT, 4), np.int64)   # global start slot of call
    call_n = np.zeros((NT, 4), np.int64)
    acc_s = 0
    for t in range(NT):
        for r in range(4):
            c0 = (t * 4 + r) * 4
            run = 0
            for j in range(4):
                if run % 128 == 96:
                    run += 32
                cell_a[c0 + j] = run
                run += int(cs[c0 + j])
            call_s[t, r] = acc_s
            call_n[t, r] = run
            acc_s += run
    TOTS = int(acc_s)                      # total slots (mult of 32)
    call_cols = (call_n + 127) // 128      # g columns per call
    gcol0 = np.zeros((NT, 4), np.int64)    # first g column of call (global)
    acc = 0
    for t in range(NT):
        for r in range(4):
            gcol0[t, r] = acc
            acc += call_cols[t, r]
    OCOLS = int(acc)

    idxflat = np.zeros((NCORES, TOTS), np.int16)
    offs = np.full((NCORES, 128, OCOLS), 999.0, np.float32)
    deg = np.zeros((NCORES, SHARD), np.float32)

    order = np.lexsort((col, cell, core))
    core_s, cell_s, off_s, col_s, rloc_s = (
        core[order], cell[order], off[order], col[order], rloc[order])
    np.add.at(deg, (core_s, rloc_s), 1.0)

    key = core_s * NCELL + cell_s
    bounds = np.flatnonzero(np.diff(key)) + 1
    starts = np.concatenate([[0], bounds])
    ends = np.concatenate([bounds, [len(key)]])
    for st, en in zip(starts, ends):
        c = int(core_s[st]); ce = int(cell_s[st]); n = en - st
        t, r = ce // 16, (ce % 16) // 4
        sl = np.arange(n) + int(cell_a[ce])            # call-local slot
        s0 = int(call_s[t, r] + cell_a[ce])
        idxflat[c, s0:s0 + n] = (col_s[st:en] // 4).astype(np.int16)
        offs[c, sl % 128, int(gcol0[t, r]) + sl // 128] = off_s[st:en]
    idx16 = np.zeros((NCORES, 128, TOTS // 16), np.int16)
    for c in range(NCORES):
        w = idxflat[c].reshape(TOTS // 16, 16).T
        idx16[c] = np.tile(w, (8, 1))
    pad_frac = (TOTS * NCORES - len(row)) / len(row)
    return dict(idx16=idx16, offs=offs.astype(BF16), deg=deg, cs=cs,
                cell_a=cell_a, call_s=call_s, call_n=call_n,
                call_cols=call_cols, gcol0=gcol0, OCOLS=OCOLS, TOTS=TOTS,
                pad_frac=pad_frac)


def _build(pp):
    import concourse.bass as bass
    import concourse.bacc as bacc
    import concourse.mybir as mybir
    import concourse.tile as tile
    from concourse.bass import IndirectOffsetOnAxis, broadcast_tensor_aps

    dt = mybir.dt
    AF = mybir.ActivationFunctionType
    OP = mybir.AluOpType
    nc = bacc.Bacc(num_devices=NCORES, num_swdge_queues=4)
    RG = [list(range(NCORES))]
    TOTS, OCOLS = pp["TOTS"], pp["OCOLS"]
    call_s, call_n = pp["call_s"], pp["call_n"]
    call_cols, gcol0 = pp["call_cols"], pp["gcol0"]
    cs, cell_a = pp["cs"], pp["cell_a"]

    hfull0 = nc.dram_tensor("hfull0", [NPAD, H], dt.bfloat16, kind="ExternalInput")
    x_T = nc.dram_tensor("x_T", [H, SHARD], dt.bfloat16, kind="ExternalInput")
    idx_d = nc.dram_tensor("idx", [128, TOTS // 16], dt.int16, kind="ExternalInput")
    offs_d = nc.dram_tensor("offs", [128, OCOLS], dt.bfloat16, kind="ExternalInput")
    deg_d = nc.dram_tensor("deg", [1, SHARD], dt.bfloat16, kind="ExternalInput")
    wct_d = nc.dram_tensor("wct", [H, 3 * H], dt.bfloat16, kind="ExternalInput")
    whht_d = nc.dram_tensor("whht", [H, 3 * H], dt.bfloat16, kind="ExternalInput")
    v3_d = nc.dram_tensor("v3", [1, 3 * H], dt.bfloat16, kind="ExternalInput")
    bias_d = nc.dram_tensor("bias", [H, 4], dt.float32, kind="ExternalInput")
    iota_d = nc.dram_tensor("iota", [H, H], dt.bfloat16, kind="ExternalInput")
    idn_d = nc.dram_tensor("idn", [H, H], dt.bfloat16, kind="ExternalInput")
    idnf_d = nc.dram_tensor("idnf", [H, H], dt.float32, kind="ExternalInput")
    out_d = nc.dram_tensor("out", [SHARD, H], dt.float32, kind="ExternalOutput")

    h_full = [nc.dram_tensor(f"h_full{i}", [NPAD, H], dt.bfloat16, kind="Internal",
                             addr_space="Shared") for i in range(2)]
    bounce = [nc.dram_tensor(f"bounce{i}", [SHARD, H], dt.bfloat16, kind="Internal")
              for i in range(2)]

    with tile.TileContext(nc) as tc:
        with (
            tc.tile_pool(name="res", bufs=1) as res,
            tc.tile_pool(name="gath", bufs=3) as gpool,
            tc.tile_pool(name="oh", bufs=3) as ohpool,
            tc.tile_pool(name="agg", bufs=2) as apool,
            tc.tile_pool(name="epi", bufs=2) as epool,
            tc.tile_pool(name="stg", bufs=2) as spool,
            tc.tile_pool(name="pseg", bufs=2, space="PSUM") as pseg,
            tc.tile_pool(name="pden", bufs=1, space="PSUM") as pden,
            tc.tile_pool(name="ptr", bufs=1, space="PSUM") as ptr,
        ):
            def ld(dram, shape, dtype, name):
                t = res.tile(shape, dtype, tag=name)
                nc.sync.dma_start(t[:], dram[:, :])
                return t

            idx_sb = ld(idx_d, [128, TOTS // 16], dt.int16, "idx")
            offs_sb = ld(offs_d, [128, OCOLS], dt.bfloat16, "offs")
            deg_sb = ld(deg_d, [1, SHARD], dt.bfloat16, "deg")
            wct = ld(wct_d, [H, 3 * H], dt.bfloat16, "wct")
            whht = ld(whht_d, [H, 3 * H], dt.bfloat16, "whht")
            v3 = ld(v3_d, [1, 3 * H], dt.bfloat16, "v3")
            bias = ld(bias_d, [H, 4], dt.float32, "bias")
            iota = ld(iota_d, [H, H], dt.bfloat16, "iota")
            idn = ld(idn_d, [H, H], dt.bfloat16, "idn")
            idnf = ld(idnf_d, [H, H], dt.float32, "idnf")

            hT = [res.tile([H, SHARD], dt.bfloat16, tag=f"hT{i}", name=f"hT{i}")
                  for i in range(2)]
            nc.sync.dma_start(hT[0][:], x_T[:, :])

            b_r = bias[:, 0:1]
            b_z = bias[:, 1:2]
            b_in = bias[:, 2:3]
            b_hn = bias[:, 3:4]

            for s in range(STEPS):
                hcur, hnxt = hT[s % 2], hT[(s + 1) % 2]
                last = s == STEPS - 1
                htab = hfull0 if s == 0 else h_full[s % 2]

                def fire_ag(ch):
                    a, b = 512 * CHB[ch], 512 * CHB[ch + 1]
                    nc.gpsimd.collective_compute(
                        "AllGather", OP.bypass, replica_groups=RG,
                        ins=[bounce[s % 2][a:b, :]],
                        outs=[h_full[(s + 1) % 2][8 * a:8 * b, :]])

                for T in range(NT):
                    if (not last and T >= AG_DELAY and (T - AG_DELAY) in CHB[1:]
                            and T - AG_DELAY < NT):
                        fire_ag(CHB.index(T - AG_DELAY) - 1)
                    gT0 = int(gcol0[T, 0])
                    CT = int(sum(call_cols[T]))
                    g = gpool.tile([128, CT, H], dt.bfloat16, tag="gath")
                    for r in range(4):
                        n_idx = int(call_n[T, r])
                        a16 = int(call_s[T, r]) // 16
                        gc = int(gcol0[T, r]) - gT0
                        nc.gpsimd.dma_gather(
                            g[:, gc:gc + int(call_cols[T, r]), :],
                            htab[r::4, :],
                            idx_sb[:, a16:a16 + n_idx // 16],
                            n_idx, n_idx, H, elem_step=4 * H, queue_num=r)
                    oh = ohpool.tile([128, CT * 128], dt.bfloat16, tag="oh")
                    a_in, b_in2 = broadcast_tensor_aps(
                        offs_sb[:, gT0:gT0 + CT, None], iota[:, None, :])
                    nc.vector.tensor_tensor(
                        oh[:].rearrange("p (c f) -> p c f", c=CT), a_in, b_in2,
                        OP.is_equal)
                    ps = pseg.tile([H, 512], dt.float32, tag="pseg")
                    for j in range(4):
                        # cell (T, r, j) occupies call-local slots [a, a+L);
                        # emit one MM per g column it touches (partition-sliced)
                        parts = []
                        for r in range(4):
                            ce = (T * 4 + r) * 4 + j
                            a = int(cell_a[ce])
                            L = int(cs[ce])
                            gc = int(gcol0[T, r]) - gT0
                            c_lo, c_hi = a // 128, (a + L - 1) // 128
                            for cc in range(c_lo, c_hi + 1):
                                p0 = max(a, cc * 128) - cc * 128
                                p1 = min(a + L, (cc + 1) * 128) - cc * 128
                                parts.append((gc + cc, p0, p1))
                        for k, (cc, p0, p1) in enumerate(parts):
                            nc.tensor.matmul(
                                ps[:, j * 128:(j + 1) * 128],
                                g[p0:p1, cc, :],
                                oh[p0:p1, cc * 128:(cc + 1) * 128],
                                start=(k == 0), stop=(k == len(parts) - 1))
                    ragg = apool.tile([H, 512], dt.bfloat16, tag="ragg")
                    nc.scalar.copy(ragg[:], ps[:])

                    hsl = hcur[:, T * 512:(T + 1) * 512]
                    dsl = deg_sb[0:1, T * 512:(T + 1) * 512]
                    p_r = pden.tile([H, 512], dt.float32, tag="p_r")
                    p_z = pden.tile([H, 512], dt.float32, tag="p_z")
                    p_in = pden.tile([H, 512], dt.float32, tag="p_in")
                    p_hn = pden.tile([H, 512], dt.float32, tag="p_hn")
                    nc.tensor.matmul(p_r[:], wct[:, 0:128], ragg[:], start=True, stop=False)
                    nc.tensor.matmul(p_r[:], whht[:, 0:128], hsl, start=False, stop=False)
                    nc.tensor.matmul(p_r[:], v3[0:1, 0:128], dsl, start=False, stop=True)
                    nc.tensor.matmul(p_z[:], wct[:, 128:256], ragg[:], start=True, stop=False)
                    nc.tensor.matmul(p_z[:], whht[:, 128:256], hsl, start=False, stop=False)
                    nc.tensor.matmul(p_z[:], v3[0:1, 128:256], dsl, start=False, stop=True)
                    nc.tensor.matmul(p_in[:], wct[:, 256:384], ragg[:], start=True, stop=False)
                    nc.tensor.matmul(p_in[:], v3[0:1, 256:384], dsl, start=False, stop=True)
                    nc.tensor.matmul(p_hn[:], whht[:, 256:384], hsl, start=True, stop=True)

                    r = epool.tile([H, 512], dt.float32, tag="r")
                    z = epool.tile([H, 512], dt.float32, tag="z")
                    ghn = epool.tile([H, 512], dt.float32, tag="ghn")
                    t2 = epool.tile([H, 512], dt.float32, tag="t2")
                    pre_n = epool.tile([H, 512], dt.float32, tag="pre_n")
                    nn = epool.tile([H, 512], dt.float32, tag="nn")
                    am = epool.tile([H, 512], dt.float32, tag="am")
                    bm = epool.tile([H, 512], dt.float32, tag="bm")

                    nc.scalar.activation(r[:], p_r[:], AF.Sigmoid, bias=b_r)
                    nc.scalar.activation(z[:], p_z[:], AF.Sigmoid, bias=b_z)
                    nc.scalar.activation(ghn[:], p_hn[:], AF.Identity, bias=b_hn)
                    nc.vector.scalar_tensor_tensor(
                        t2[:], r[:], 0.0, ghn[:], OP.add, OP.mult)
                    nc.vector.tensor_tensor(pre_n[:], t2[:], p_in[:], OP.add)
                    nc.scalar.activation(nn[:], pre_n[:], AF.Tanh, bias=b_in)
                    nc.vector.tensor_tensor(am[:], hsl, nn[:], OP.subtract)
                    nc.vector.scalar_tensor_tensor(
                        bm[:], z[:], 0.0, am[:], OP.add, OP.mult)

                    if not last:
                        hn_sl = hnxt[:, T * 512:(T + 1) * 512]
                        nc.vector.tensor_tensor(hn_sl, bm[:], nn[:], OP.add)
                        stg = spool.tile([128, 4, H], dt.bfloat16, tag="stg")
                        for j in range(4):
                            pt = ptr.tile([128, 128], dt.bfloat16, tag="pt")
                            nc.tensor.transpose(
                                pt[:], hnxt[:, T * 512 + j * 128: T * 512 + (j + 1) * 128],
                                idn[:])
                            nc.scalar.copy(stg[:, j, :], pt[:])
                        nc.sync.dma_start(
                            bounce[s % 2].rearrange(
                                "(t g p) f -> t p g f", p=128, g=4)[T],
                            stg[:])
                    else:
                        hf = epool.tile([H, 512], dt.float32, tag="hf", bufs=2)
                        nc.vector.tensor_tensor(hf[:], bm[:], nn[:], OP.add)
                        stgf = spool.tile([128, 4, H], dt.float32, tag="stgf")
                        for j in range(4):
                            ptf = ptr.tile([128, 128], dt.float32, tag="ptf")
                            nc.tensor.matmul(ptf[:], hf[:, j * 128:(j + 1) * 128],
                                             idnf[:], is_transpose=True)
                            nc.scalar.copy(stgf[:, j, :], ptf[:])
                        nc.sync.dma_start(
                            out_d.rearrange("(t g p) f -> t p g f", p=128, g=4)[T],
                            stgf[:])
                if not last:
                    for ch in range(NCH):
                        if CHB[ch + 1] + AG_DELAY >= NT:
                            fire_ag(ch)
    nc.finalize()
    return nc


_CACHE = {}


def kernel(**inputs):
    x = np.asarray(inputs["x"], np.float32)
    edge_index = np.asarray(inputs["edge_index"])
    W_msg = np.asarray(inputs["W_msg"], np.float32)
    b_msg = np.asarray(inputs["b_msg"], np.float32)
    W_ih = np.asarray(inputs["W_ih"], np.float32)
    W_hh = np.asarray(inputs["W_hh"], np.float32)
    b_ih = np.asarray(inputs["b_ih"], np.float32)
    b_hh = np.asarray(inputs["b_hh"], np.float32)

    pp = _preprocess(edge_index)
    key = (pp["TOTS"], tuple(pp["cs"].tolist()))
    if key not in _CACHE:
        _CACHE[key] = _build(pp)
    nc = _CACHE[key]

    xp = np.zeros((NPAD, H), np.float32)
    xp[:N] = x
    xperm = np.empty((NPAD, H), np.float32)
    xperm[_perm()] = xp
    xperm_bf = xperm.astype(BF16)
    W_c = W_ih @ W_msg
    v3 = (W_ih @ b_msg).reshape(1, 3 * H)
    bias = np.stack([
        b_ih[0:128] + b_hh[0:128],
        b_ih[128:256] + b_hh[128:256],
        b_ih[256:384],
        b_hh[256:384],
    ], axis=1).astype(np.float32)
    iota = np.broadcast_to(np.arange(H, dtype=np.float32), (H, H)).astype(BF16)
    idn = np.eye(H, dtype=np.float32)

    in_maps = []
    for c in range(NCORES):
        sh = xp[c * SHARD:(c + 1) * SHARD]
        in_maps.append({
            "hfull0": xperm_bf,
            "x_T": np.ascontiguousarray(sh.T).astype(BF16),
            "idx": pp["idx16"][c],
            "offs": pp["offs"][c],
            "deg": pp["deg"][c].reshape(1, SHARD).astype(BF16),
            "wct": np.ascontiguousarray(W_c.T).astype(BF16),
            "whht": np.ascontiguousarray(W_hh.T).astype(BF16),
            "v3": v3.astype(BF16),
            "bias": bias,
            "iota": np.ascontiguousarray(iota),
            "idn": idn.astype(BF16),
            "idnf": idn,
        })

    global _last_in_maps
    _last_in_maps = in_maps
    from concourse.bass_utils import run_bass_kernel_spmd
    res = run_bass_kernel_spmd(nc, in_maps, core_ids=list(range(NCORES)))
    outs = res.results
    full = np.concatenate([outs[c]["out"] for c in range(NCORES)], axis=0)
    return full[:N].astype(np.float32)



# revision 50
# speedup vs baseline: 1.0718x; 1.0718x over previous
"""GatedGraphNN Trainium2 kernel: 8-core SPMD, node-sharded, overlapped AllGather.

Algorithm notes:
  - messages = h[col] @ W_msg.T + b_msg ; agg = segsum(messages, row). Linearity:
    agg @ W_ih.T = raw @ (W_ih W_msg).T + outer(deg, W_ih b_msg), raw = segsum(h[col]).
    So no per-edge matmul: gather h[col] (bf16), segment-sum via one-hot matmuls on PE,
    then fused dense GRU with W_c = W_ih @ W_msg.
  - Each core owns 12800 dest nodes (N padded 100000->102400). Edges bucketed by
    (dest-tile, src-residue, dest-quarter) cells padded to 32-slot granularity (size
    shared across cores: SPMD-uniform program); segsum matmuls partition-slice cells
    that cross 128-slot g columns (PE base partition must be 0/32/64, so cell starts
    avoid 96 mod 128 via spacers). ~21% slot padding vs 36% at 128 granularity.
  - Gathers use 4 SWDGE queues (queue_num = source residue r): descriptor generation
    runs on 4 Q7 core-pairs concurrently, ~3x the single-queue rate.
  - h kept bf16: node-major replica h_full [102400,128] in DRAM for gathers (step 0
    reads the host-provided full x, so there is no initial exchange); feature-major
    shard resident in SBUF for the W_hh matmul.
  - The per-step AllGather is split into geometric chunks (CHB) fired as their dest
    tiles finish, overlapping the exchange with compute; h_full and bounce are
    double-buffered across steps to avoid gather-vs-exchange races. Node rows are
    permuted chunk-major (see _perm) so each chunk's output is contiguous.
  - Epilogue (gates, blend) computed with fp32 intermediates; h state rounds to bf16
    once per step. Final step emits fp32.
"""

import numpy as np
import ml_dtypes

BF16 = ml_dtypes.bfloat16
N, H, STEPS, NCORES = 100000, 128, 5, 8
NPAD = 102400
SHARD = NPAD // NCORES          # 12800
NT = SHARD // 512               # 25 dense tiles of 512 dests
NQ = SHARD // 128               # 100 quarters of 128 dests
CHUNK = 128
# AllGather chunk boundaries (in dest tiles): decreasing sizes so late chunks
# still hide under remaining compute; h_full rows are chunk-major then core.
# Each chunk's AllGather is issued 2 tiles after the chunk completes so the
# issue's sem wait (on bounce writes) doesn't stall the gather stream.
CHB = [0, 2, 6, 11, 17, 25]
NCH = len(CHB) - 1
AG_DELAY = 1


def _perm():
    """P[n] = physical row of logical node n in h_full (chunk-major layout)."""
    n = np.arange(NPAD, dtype=np.int64)
    c, l = n // SHARD, n % SHARD
    t, o = l // 512, l % 512
    chb = np.asarray(CHB)
    ch = np.searchsorted(chb, t, side="right") - 1
    base = 8 * 512 * chb[ch]
    size = 512 * (chb[ch + 1] - chb[ch])
    return base + c * size + (t - chb[ch]) * 512 + o


def _preprocess(edge_index):
    """Host-side tables, 32-granular cells. Slots grouped per gather call (T, r)
    with the 4 dest-qtr cells at 32-aligned offsets inside the call; no 128
    alignment, so segsum matmuls partition-slice cells that cross g columns.
    idx value = pcol//4 (stride-4 table view), pcol = permuted source row."""
    row = np.asarray(edge_index[0]).astype(np.int64)
    col = np.asarray(edge_index[1]).astype(np.int64)
    col = _perm()[col]
    core = row // SHARD
    rloc = row - core * SHARD
    T = rloc // 512
    dq = (rloc % 512) // 128
    off = rloc % 128
    res = col % 4
    cell = (T * 4 + res) * 4 + dq          # (T, res, dq) -> 0..399
    NCELL = NT * 16
    cnt = np.zeros((NCORES, NCELL), np.int64)
    np.add.at(cnt, (core, cell), 1)
    cs = np.maximum(32, ((cnt.max(axis=0) + 31) // 32) * 32)   # slots per cell
    # cell start offsets within each call; PE base-partition must be 0/32/64,
    # so a cell may not start at 96 mod 128 -> insert a 32-slot spacer there
    cell_a = np.zeros(NCELL, np.int64)     # call-local start of each cell
    call_s = np.zeros((NT, 4), np.int64)   # global start slot of call
    call_n = np.zeros((NT, 4), np.int64)
    acc_s = 0
    for t in range(NT):
        for r in range(4):
            c0 = (t * 4 + r) * 4
            run = 0
            for j in range(4):
                if run % 128 == 96:
                    run += 32
                cell_a[c0 + j] = run
                run += int(cs[c0 + j])
            call_s[t, r] = acc_s
            call_n[t, r] = run
            acc_s += run
    TOTS = int(acc_s)                      # total slots (mult of 32)
    call_cols = (call_n + 127) // 128      # g columns per call
    gcol0 = np.zeros((NT, 4), np.int64)    # first g column of call (global)
    acc = 0
    for t in range(NT):
        for r in range(4):
            gcol0[t, r] = acc
            acc += call_cols[t, r]
    OCOLS = int(acc)

    idxflat = np.zeros((NCORES, TOTS), np.int16)
    offs = np.full((NCORES, 128, OCOLS), 999.0, np.float32)
    deg = np.zeros((NCORES, SHARD), np.float32)

    order = np.lexsort((col, cell, core))
    core_s, cell_s, off_s, col_s, rloc_s = (
        core[order], cell[order], off[order], col[order], rloc[order])
    np.add.at(deg, (core_s, rloc_s), 1.0)

    key = core_s * NCELL + cell_s
    bounds = np.flatnonzero(np.diff(key)) + 1
    starts = np.concatenate([[0], bounds])
    ends = np.concatenate([bounds, [len(key)]])
    for st, en in zip(starts, ends):
        c = int(core_s[st]); ce = int(cell_s[st]); n = en - st
        t, r = ce // 16, (ce % 16) // 4
        sl = np.arange(n) + int(cell_a[ce])            # call-local slot
        s0 = int(call_s[t, r] + cell_a[ce])
        idxflat[c, s0:s0 + n] = (col_s[st:en] // 4).astype(np.int16)
        offs[c, sl % 128, int(gcol0[t, r]) + sl // 128] = off_s[st:en]
    idx16 = np.zeros((NCORES, 128, TOTS // 16), np.int16)
    for c in range(NCORES):
        w = idxflat[c].reshape(TOTS // 16, 16).T
        idx16[c] = np.tile(w, (8, 1))
    pad_frac = (TOTS * NCORES - len(row)) / len(row)
    return dict(idx16=idx16, offs=offs.astype(BF16), deg=deg, cs=cs,
                cell_a=cell_a, call_s=call_s, call_n=call_n,
                call_cols=call_cols, gcol0=gcol0, OCOLS=OCOLS, TOTS=TOTS,
                pad_frac=pad_frac)


def _build(pp):
    import concourse.bass as bass
    import concourse.bacc as bacc
    import concourse.mybir as mybir
    import concourse.tile as tile
    from concourse.bass import IndirectOffsetOnAxis, broadcast_tensor_aps

    dt = mybir.dt
    AF = mybir.ActivationFunctionType
    OP = mybir.AluOpType
    nc = bacc.Bacc(num_devices=NCORES, num_swdge_queues=4)
    RG = [list(range(NCORES))]
    TOTS, OCOLS = pp["TOTS"], pp["OCOLS"]
    call_s, call_n = pp["call_s"], pp["call_n"]
    call_cols, gcol0 = pp["call_cols"], pp["gcol0"]
    cs, cell_a = pp["cs"], pp["cell_a"]

    hfull0 = nc.dram_tensor("hfull0", [NPAD, H], dt.bfloat16, kind="ExternalInput")
    x_T = nc.dram_tensor("x_T", [H, SHARD], dt.bfloat16, kind="ExternalInput")
    idx_d = nc.dram_tensor("idx", [128, TOTS // 16], dt.int16, kind="ExternalInput")
    offs_d = nc.dram_tensor("offs", [128, OCOLS], dt.bfloat16, kind="ExternalInput")
    deg_d = nc.dram_tensor("deg", [1, SHARD], dt.bfloat16, kind="ExternalInput")
    wct_d = nc.dram_tensor("wct", [H, 3 * H], dt.bfloat16, kind="ExternalInput")
    whht_d = nc.dram_tensor("whht", [H, 3 * H], dt.bfloat16, kind="ExternalInput")
    v3_d = nc.dram_tensor("v3", [1, 3 * H], dt.bfloat16, kind="ExternalInput")
    bias_d = nc.dram_tensor("bias", [H, 4], dt.float32, kind="ExternalInput")
    iota_d = nc.dram_tensor("iota", [H, H], dt.bfloat16, kind="ExternalInput")
    idn_d = nc.dram_tensor("idn", [H, H], dt.bfloat16, kind="ExternalInput")
    idnf_d = nc.dram_tensor("idnf", [H, H], dt.float32, kind="ExternalInput")
    out_d = nc.dram_tensor("out", [SHARD, H], dt.float32, kind="ExternalOutput")

    h_full = [nc.dram_tensor(f"h_full{i}", [NPAD, H], dt.bfloat16, kind="Internal",
                             addr_space="Shared") for i in range(2)]
    bounce = [nc.dram_tensor(f"bounce{i}", [SHARD, H], dt.bfloat16, kind="Internal")
              for i in range(2)]

    with tile.TileContext(nc) as tc:
        with (
            tc.tile_pool(name="res", bufs=1) as res,
            tc.tile_pool(name="gath", bufs=3) as gpool,
            tc.tile_pool(name="oh", bufs=3) as ohpool,
            tc.tile_pool(name="agg", bufs=2) as apool,
            tc.tile_pool(name="epi", bufs=2) as epool,
            tc.tile_pool(name="stg", bufs=2) as spool,
            tc.tile_pool(name="pseg", bufs=2, space="PSUM") as pseg,
            tc.tile_pool(name="pden", bufs=1, space="PSUM") as pden,
            tc.tile_pool(name="ptr", bufs=1, space="PSUM") as ptr,
        ):
            def ld(dram, shape, dtype, name):
                t = res.tile(shape, dtype, tag=name)
                nc.sync.dma_start(t[:], dram[:, :])
                return t

            idx_sb = ld(idx_d, [128, TOTS // 16], dt.int16, "idx")
            offs_sb = ld(offs_d, [128, OCOLS], dt.bfloat16, "offs")
            deg_sb = ld(deg_d, [1, SHARD], dt.bfloat16, "deg")
            wct = ld(wct_d, [H, 3 * H], dt.bfloat16, "wct")
            whht = ld(whht_d, [H, 3 * H], dt.bfloat16, "whht")
            v3 = ld(v3_d, [1, 3 * H], dt.bfloat16, "v3")
            bias = ld(bias_d, [H, 4], dt.float32, "bias")
            iota = ld(iota_d, [H, H], dt.bfloat16, "iota")
            idn = ld(idn_d, [H, H], dt.bfloat16, "idn")
            idnf = ld(idnf_d, [H, H], dt.float32, "idnf")

            hT = [res.tile([H, SHARD], dt.bfloat16, tag=f"hT{i}", name=f"hT{i}")
                  for i in range(2)]
            nc.sync.dma_start(hT[0][:], x_T[:, :])

            b_r = bias[:, 0:1]
            b_z = bias[:, 1:2]
            b_in = bias[:, 2:3]
            b_hn = bias[:, 3:4]

            for s in range(STEPS):
                hcur, hnxt = hT[s % 2], hT[(s + 1) % 2]
                last = s == STEPS - 1
                htab = hfull0 if s == 0 else h_full[s % 2]

                def fire_ag(ch):
                    a, b = 512 * CHB[ch], 512 * CHB[ch + 1]
                    nc.gpsimd.collective_compute(
                        "AllGather", OP.bypass, replica_groups=RG,
                        ins=[bounce[s % 2][a:b, :]],
                        outs=[h_full[(s + 1) % 2][8 * a:8 * b, :]])

                for T in range(NT):
                    if (not last and T >= AG_DELAY and (T - AG_DELAY) in CHB[1:]
                            and T - AG_DELAY < NT):
                        fire_ag(CHB.index(T - AG_DELAY) - 1)
                    gT0 = int(gcol0[T, 0])
                    CT = int(sum(call_cols[T]))
                    g = gpool.tile([128, CT, H], dt.bfloat16, tag="gath")
                    for r in range(4):
                        n_idx = int(call_n[T, r])
                        a16 = int(call_s[T, r]) // 16
                        gc = int(gcol0[T, r]) - gT0
                        nc.gpsimd.dma_gather(
                            g[:, gc:gc + int(call_cols[T, r]), :],
                            htab[r::4, :],
                            idx_sb[:, a16:a16 + n_idx // 16],
                            n_idx, n_idx, H, elem_step=4 * H, queue_num=r)
                    oh = ohpool.tile([128, CT * 128], dt.bfloat16, tag="oh")
                    a_in, b_in2 = broadcast_tensor_aps(
                        offs_sb[:, gT0:gT0 + CT, None], iota[:, None, :])
                    nc.vector.tensor_tensor(
                        oh[:].rearrange("p (c f) -> p c f", c=CT), a_in, b_in2,
                        OP.is_equal)
                    ps = pseg.tile([H, 512], dt.float32, tag="pseg")
                    for j in range(4):
                        # cell (T, r, j) occupies call-local slots [a, a+L);
                        # emit one MM per g column it touches (partition-sliced)
                        parts = []
                        for r in range(4):
                            ce = (T * 4 + r) * 4 + j
                            a = int(cell_a[ce])
                            L = int(cs[ce])
                            gc = int(gcol0[T, r]) - gT0
                            c_lo, c_hi = a // 128, (a + L - 1) // 128
                            for cc in range(c_lo, c_hi + 1):
                                p0 = max(a, cc * 128) - cc * 128
                                p1 = min(a + L, (cc + 1) * 128) - cc * 128
                                parts.append((gc + cc, p0, p1))
                        for k, (cc, p0, p1) in enumerate(parts):
                            nc.tensor.matmul(
                                ps[:, j * 128:(j + 1) * 128],
                                g[p0:p1, cc, :],
                                oh[p0:p1, cc * 128:(cc + 1) * 128],
                                start=(k == 0), stop=(k == len(parts) - 1))
                    ragg = apool.tile([H, 512], dt.bfloat16, tag="ragg")
                    nc.scalar.copy(ragg[:], ps[:])

                    hsl = hcur[:, T * 512:(T + 1) * 512]
                    dsl = deg_sb[0:1, T * 512:(T + 1) * 512]
                    p_r = pden.tile([H, 512], dt.float32, tag="p_r")
                    p_z = pden.tile([H, 512], dt.float32, tag="p_z")
                    p_in = pden.tile([H, 512], dt.float32, tag="p_in")
                    p_hn = pden.tile([H, 512], dt.float32, tag="p_hn")
                    nc.tensor.matmul(p_r[:], wct[:, 0:128], ragg[:], start=True, stop=False)
                    nc.tensor.matmul(p_r[:], whht[:, 0:128], hsl, start=False, stop=False)
                    nc.tensor.matmul(p_r[:], v3[0:1, 0:128], dsl, start=False, stop=True)
                    nc.tensor.matmul(p_z[:], wct[:, 128:256], ragg[:], start=True, stop=False)
                    nc.tensor.matmul(p_z[:], whht[:, 128:256], hsl, start=False, stop=False)
                    nc.tensor.matmul(p_z[:], v3[0:1, 128:256], dsl, start=False, stop=True)
                    nc.tensor.matmul(p_in[:], wct[:, 256:384], ragg[:], start=True, stop=False)
                    nc.tensor.matmul(p_in[:], v3[0:1, 256:384], dsl, start=False, stop=True)
                    nc.tensor.matmul(p_hn[:], whht[:, 256:384], hsl, start=True, stop=True)

                    r = epool.tile([H, 512], dt.float32, tag="r")
                    z = epool.tile([H, 512], dt.float32, tag="z")
                    ghn = epool.tile([H, 512], dt.float32, tag="ghn")
                    t2 = epool.tile([H, 512], dt.float32, tag="t2")
                    pre_n = epool.tile([H, 512], dt.float32, tag="pre_n")
                    nn = epool.tile([H, 512], dt.float32, tag="nn")
                    am = epool.tile([H, 512], dt.float32, tag="am")
                    bm = epool.tile([H, 512], dt.float32, tag="bm")

                    nc.scalar.activation(r[:], p_r[:], AF.Sigmoid, bias=b_r)
                    nc.scalar.activation(z[:], p_z[:], AF.Sigmoid, bias=b_z)
                    nc.scalar.activation(ghn[:], p_hn[:], AF.Identity, bias=b_hn)
                    nc.vector.scalar_tensor_tensor(
                        t2[:], r[:], 0.0, ghn[:], OP.add, OP.mult)
                    nc.vector.tensor_tensor(pre_n[:], t2[:], p_in[:], OP.add)
                    nc.scalar.activation(nn[:], pre_n[:], AF.Tanh, bias=b_in)
                    nc.vector.tensor_tensor(am[:], hsl, nn[:], OP.subtract)
                    nc.vector.scalar_tensor_tensor(
                        bm[:], z[:], 0.0, am[:], OP.add, OP.mult)

                    if not last:
                        hn_sl = hnxt[:, T * 512:(T + 1) * 512]
                        nc.vector.tensor_tensor(hn_sl, bm[:], nn[:], OP.add)
                        stg = spool.tile([128, 4, H], dt.bfloat16, tag="stg")
                        for j in range(4):
                            pt = ptr.tile([128, 128], dt.bfloat16, tag="pt")
                            nc.tensor.transpose(
                                pt[:], hnxt[:, T * 512 + j * 128: T * 512 + (j + 1) * 128],
                                idn[:])
                            nc.scalar.copy(stg[:, j, :], pt[:])
                        nc.sync.dma_start(
                            bounce[s % 2].rearrange(
                                "(t g p) f -> t p g f", p=128, g=4)[T],
                            stg[:])
                    else:
                        hf = epool.tile([H, 512], dt.float32, tag="hf", bufs=2)
                        nc.vector.tensor_tensor(hf[:], bm[:], nn[:], OP.add)
                        stgf = spool.tile([128, 4, H], dt.float32, tag="stgf")
                        for j in range(4):
                            ptf = ptr.tile([128, 128], dt.float32, tag="ptf")
                            nc.tensor.matmul(ptf[:], hf[:, j * 128:(j + 1) * 128],
                                             idnf[:], is_transpose=True)
                            nc.scalar.copy(stgf[:, j, :], ptf[:])
                        nc.sync.dma_start(
                            out_d.rearrange("(t g p) f -> t p g f", p=128, g=4)[T],
                            stgf[:])
                if not last:
                    for ch in range(NCH):
                        if CHB[ch + 1] + AG_DELAY >= NT:
                            fire_ag(ch)
    nc.finalize()
    return nc


_CACHE = {}


def kernel(**inputs):
    x = np.asarray(inputs["x"], np.float32)
    edge_index = np.asarray(inputs["edge_index"])
    W_msg = np.asarray(inputs["W_msg"], np.float32)
    b_msg = np.asarray(inputs["b_msg"], np.float32)
    W_ih = np.asarray(inputs["W_ih"], np.float32)
    W_hh = np.asarray(inputs["W_hh"], np.float32)
    b_ih = np.asarray(inputs["b_ih"], np.float32)
    b_hh = np.asarray(inputs["b_hh"], np.float32)

    pp = _preprocess(edge_index)
    key = (pp["TOTS"], tuple(pp["cs"].tolist()))
    if key not in _CACHE:
        _CACHE[key] = _build(pp)
    nc = _CACHE[key]

    xp = np.zeros((NPAD, H), np.float32)
    xp[:N] = x
    xperm = np.empty((NPAD, H), np.float32)
    xperm[_perm()] = xp
    xperm_bf = xperm.astype(BF16)
    W_c = W_ih @ W_msg
    v3 = (W_ih @ b_msg).reshape(1, 3 * H)
    bias = np.stack([
        b_ih[0:128] + b_hh[0:128],
        b_ih[128:256] + b_hh[128:256],
        b_ih[256:384],
        b_hh[256:384],
    ], axis=1).astype(np.float32)
    iota = np.broadcast_to(np.arange(H, dtype=np.float32), (H, H)).astype(BF16)
    idn = np.eye(H, dtype=np.float32)

    in_maps = []
    for c in range(NCORES):
        sh = xp[c * SHARD:(c + 1) * SHARD]
        in_maps.append({
            "hfull0": xperm_bf,
            "x_T": np.ascontiguousarray(sh.T).astype(BF16),
            "idx": pp["idx16"][c],
            "offs": pp["offs"][c],
            "deg": pp["deg"][c].reshape(1, SHARD).astype(BF16),
            "wct": np.ascontiguousarray(W_c.T).astype(BF16),
            "whht": np.ascontiguousarray(W_hh.T).astype(BF16),
            "v3": v3.astype(BF16),
            "bias": bias,
            "iota": np.ascontiguousarray(iota),
            "idn": idn.astype(BF16),
            "idnf": idn,
        })

    global _last_in_maps
    _last_in_maps = in_maps
    from concourse.bass_utils import run_bass_kernel_spmd
    res = run_bass_kernel_spmd(nc, in_maps, core_ids=list(range(NCORES)))
    outs = res.results
    full = np.concatenate([outs[c]["out"] for c in range(NCORES)], axis=0)
    return full[:N].astype(np.float32)

